# revision 8
# baseline (speedup 1.0000x reference)
"""GAT (graph attention) Bass kernel for Trainium2, 8-core SPMD.

Strategy: edge-parallel with receiver-range sharding. Host sorts edges by
receiver and packs receivers into fixed-capacity "windows" (<=127 nodes,
<=G*128 edges). Each core processes an equal number of windows; the segment
softmax and weighted segment-sum are fully core-local (no collectives).

Device kernel, per core:
  phase A: hs = [x@W | x@W@A1 | x@W@A2]  (A1/A2 embed the per-head attention
           vectors) -> DRAM tables hs[N,68] (h|s1) and s2[N,4].
  phase B: per window, indirect-DMA gather of the 2048 edge rows (by sender
           for h|s1, by receiver for s2), logits -> LeakyReLU -> exp on the
           scalar engine, feature scaling + one-hot build on the vector
           engine, 16 accumulating 128-contraction matmuls into PSUM
           (segment sum of both softmax numerator and denominator), then a
           reciprocal multiply and a contiguous DMA to a staged output.

Host reassembles the staged windows into the full [N, H*U] output.
"""

import os
import sys

import numpy as np

for _p in ("/opt/trn_rl_repo", os.path.expanduser("~/.axon_site/_ro/trn_rl_repo")):
    if os.path.isdir(_p) and _p not in sys.path:
        sys.path.insert(0, _p)

P = 128          # partitions / PE contraction
G = 16           # edge groups per window (window = G*128 edge slots)
WIN_EDGES = G * P
WIN_NODES = 127  # real receiver rows per window; row 127 collects pad edges
HEADS = 4
UNITS = 16
HU = HEADS * UNITS          # 64
HS_COLS = HU + HEADS        # 68: h | s1
LEAKY_ALPHA = 0.2
XTILE = 512                 # phase-A node tile


def _pack_windows(rcv_sorted, order, n_nodes):
    """Pack receivers (ascending) into windows of <=WIN_NODES nodes and
    <=WIN_EDGES edges. Returns per-window (node_base, node_count, edge slice
    into `order`)."""
    deg = np.bincount(rcv_sorted, minlength=n_nodes)
    starts = np.concatenate(([0], np.cumsum(deg)))
    windows = []
    n = 0
    while n < n_nodes:
        n0 = n
        e0 = starts[n]
        while (
            n < n_nodes
            and (n - n0) < WIN_NODES
            and (starts[n + 1] - e0) <= WIN_EDGES
        ):
            n += 1
        assert n > n0, f"node {n} degree {deg[n]} exceeds window capacity"
        windows.append((n0, n - n0, e0, starts[n]))
    return windows


def _build_host_data(x, edge_index, W, att_w1, att_w2, n_cores):
    n_nodes, in_feat = x.shape
    snd = edge_index[:, 0].astype(np.int64)
    rcv = edge_index[:, 1].astype(np.int64)

    order = np.argsort(rcv, kind="stable")
    rcv_sorted = rcv[order]
    windows = _pack_windows(rcv_sorted, order, n_nodes)

    nw_total = len(windows)
    nw = -(-nw_total // n_cores)  # windows per core, padded
    n_win_padded = nw * n_cores

    # per-window device metadata, edge slot q=(p*G+j) <- position q of the
    # window's (padded) edge list
    snd_idx = np.zeros((n_win_padded, P, G), dtype=np.int32)
    s2_idx = np.zeros((n_win_padded, P, G), dtype=np.int32)
    rcv_loc = np.full((n_win_padded, P, G), float(P - 1), dtype=np.float32)

    for w, (n0, cnt, e0, e1) in enumerate(windows):
        ne = e1 - e0
        eidx = order[e0:e1]
        buf_s = np.zeros(WIN_EDGES, dtype=np.int32)
        buf_r = np.full(WIN_EDGES, float(P - 1), dtype=np.float32)
        buf_v = np.zeros(WIN_EDGES, dtype=np.int32)
        buf_s[:ne] = snd[eidx]
        buf_r[:ne] = (rcv_sorted[e0:e1] - n0).astype(np.float32)
        buf_v[:ne] = rcv_sorted[e0:e1]
        snd_idx[w] = buf_s.reshape(P, G)
        rcv_loc[w] = buf_r.reshape(P, G)
        s2_idx[w] = buf_v.reshape(P, G)

    # attention vectors as [HU, 2H]: A[h*U+u, h] = att_w1[h,0,u]; +H col for w2
    A12 = np.zeros((HU, 2 * HEADS), dtype=np.float32)
    for h in range(HEADS):
        A12[h * UNITS:(h + 1) * UNITS, h] = att_w1[h, 0]
        A12[h * UNITS:(h + 1) * UNITS, HEADS + h] = att_w2[h, 0]

    npad = -(-n_nodes // XTILE) * XTILE
    xT = np.zeros((in_feat, npad), dtype=np.float32)
    xT[:, :n_nodes] = np.ascontiguousarray(x.T)

    iota = np.broadcast_to(np.arange(P, dtype=np.float32), (P, P)).copy()
    identity = np.eye(P, dtype=np.float32)

    host = {
        "windows": windows,
        "nw": nw,
        "npad": npad,
        "deg": np.bincount(rcv, minlength=n_nodes),
    }
    per_core = []
    for c in range(n_cores):
        w0 = c * nw
        per_core.append({
            "xT": xT,
            "W": np.ascontiguousarray(W.astype(np.float32)),
            "A12": A12,
            "iota": iota,
            "identity": identity,
            "snd_idx": np.ascontiguousarray(
                snd_idx[w0:w0 + nw].transpose(1, 0, 2).reshape(P, nw * G)),
            "rcv_loc": np.ascontiguousarray(
                rcv_loc[w0:w0 + nw].transpose(1, 0, 2).reshape(P, nw * G)),
            "s2_idx": np.ascontiguousarray(
                s2_idx[w0:w0 + nw].transpose(1, 0, 2).reshape(P, nw * G)),
        })
    return host, per_core


def _build_bass(n_nodes, npad, nw, in_feat):
    from concourse import bacc, mybir, tile
    import concourse.bass as bass

    f32 = mybir.dt.float32
    i32 = mybir.dt.int32

    nc = bacc.Bacc("TRN2", target_bir_lowering=False, debug=False,
                   enable_asserts=False, num_devices=1)

    xT_d = nc.dram_tensor("xT", [in_feat, npad], f32, kind="ExternalInput").ap()
    W_d = nc.dram_tensor("W", [in_feat, HU], f32, kind="ExternalInput").ap()
    A12_d = nc.dram_tensor("A12", [HU, 2 * HEADS], f32, kind="ExternalInput").ap()
    iota_d = nc.dram_tensor("iota", [P, P], f32, kind="ExternalInput").ap()
    ident_d = nc.dram_tensor("identity", [P, P], f32, kind="ExternalInput").ap()
    snd_d = nc.dram_tensor("snd_idx", [P, nw * G], i32, kind="ExternalInput").ap()
    rcvl_d = nc.dram_tensor("rcv_loc", [P, nw * G], f32, kind="ExternalInput").ap()
    s2i_d = nc.dram_tensor("s2_idx", [P, nw * G], i32, kind="ExternalInput").ap()

    out_d = nc.dram_tensor("staged", [nw * P, HU], f32, kind="ExternalOutput").ap()

    debug = bool(os.environ.get("GAT_DEBUG"))
    tab_kind = "ExternalOutput" if debug else "Internal"
    hs_d = nc.dram_tensor("hs_tab", [npad, HS_COLS], f32, kind=tab_kind).ap()
    s2_d = nc.dram_tensor("s2_tab", [npad, HEADS], f32, kind=tab_kind).ap()
    if debug:
        dbg_hsg = nc.dram_tensor("dbg_hsg", [P, G * HS_COLS], f32,
                                 kind="ExternalOutput").ap()
        dbg_oh = nc.dram_tensor("dbg_oh", [P, G * P], f32,
                                kind="ExternalOutput").ap()
        dbg_rhs = nc.dram_tensor("dbg_rhs", [P, G * HS_COLS], f32,
                                 kind="ExternalOutput").ap()
        dbg_ps = nc.dram_tensor("dbg_ps", [P, HS_COLS], f32,
                                kind="ExternalOutput").ap()

    ntiles = npad // XTILE

    with tile.TileContext(nc) as tc:
        with tc.tile_pool(name="consts", bufs=1) as cpool:
            W_sb = cpool.tile([in_feat, HU], f32, tag="w")
            nc.sync.dma_start(out=W_sb[:], in_=W_d[:])
            A12_sb = cpool.tile([HU, 2 * HEADS], f32, tag="a12")
            nc.sync.dma_start(out=A12_sb[:], in_=A12_d[:])
            iota_sb = cpool.tile([P, P], f32, tag="iota")
            nc.sync.dma_start(out=iota_sb[:], in_=iota_d[:])
            id_sb = cpool.tile([P, P], f32, tag="ident")
            nc.sync.dma_start(out=id_sb[:], in_=ident_d[:])
            snd_sb = cpool.tile([P, nw * G], i32, tag="snd")
            nc.sync.dma_start(out=snd_sb[:], in_=snd_d[:])
            rcvl_sb = cpool.tile([P, nw * G], f32, tag="rcvl")
            nc.sync.dma_start(out=rcvl_sb[:], in_=rcvl_d[:])
            s2i_sb = cpool.tile([P, nw * G], i32, tag="s2i")
            nc.sync.dma_start(out=s2i_sb[:], in_=s2i_d[:])
            wcat_sb = cpool.tile([in_feat, HU + 2 * HEADS], f32, tag="wcat")

            # fold attention vectors: WA = W @ A12 (needs W^T as lhsT)
            with tc.tile_pool(name="p0psum", bufs=1, space="PSUM") as p0:
                wt_ps = p0.tile([HU, in_feat], f32, tag="wt")
                nc.tensor.transpose(out=wt_ps[:], in_=W_sb[:], identity=id_sb[:])
                wt_sb = cpool.tile([HU, in_feat], f32, tag="wtsb")
                nc.vector.tensor_copy(out=wt_sb[:], in_=wt_ps[:])
                wa_ps = p0.tile([in_feat, 2 * HEADS], f32, tag="wa")
                nc.tensor.matmul(out=wa_ps[:], lhsT=wt_sb[:], rhs=A12_sb[:],
                                 start=True, stop=True)
                nc.vector.tensor_copy(out=wcat_sb[:, HU:], in_=wa_ps[:])
                nc.vector.tensor_copy(out=wcat_sb[:, :HU], in_=W_sb[:])

            # ---- phase A: hs tables ----
            nblk = XTILE // P
            wc = HU + 2 * HEADS  # 72
            with tc.tile_pool(name="pa_x", bufs=3) as pax, \
                 tc.tile_pool(name="pa_ps", bufs=2, space="PSUM") as paps, \
                 tc.tile_pool(name="pa_hs", bufs=3) as pahs:
                for t in range(ntiles):
                    xt = pax.tile([in_feat, XTILE], f32, tag="xt")
                    nc.sync.dma_start(
                        out=xt[:], in_=xT_d[:, t * XTILE:(t + 1) * XTILE])
                    ps = paps.tile([P, nblk * wc], f32, tag="ps")
                    for i in range(nblk):
                        nc.tensor.matmul(
                            out=ps[:, i * wc:(i + 1) * wc],
                            lhsT=xt[:, i * P:(i + 1) * P],
                            rhs=wcat_sb[:], start=True, stop=True)
                    hsb = pahs.tile([P, nblk * wc], f32, tag="hsb")
                    nc.vector.tensor_copy(out=hsb[:], in_=ps[:])
                    hsb3 = hsb[:].rearrange("p (i c) -> p i c", c=wc)
                    dst_hs = hs_d[t * XTILE:(t + 1) * XTILE, :].rearrange(
                        "(i p) c -> p i c", p=P)
                    nc.sync.dma_start(out=dst_hs, in_=hsb3[:, :, 0:HS_COLS])
                    dst_s2 = s2_d[t * XTILE:(t + 1) * XTILE, :].rearrange(
                        "(i p) c -> p i c", p=P)
                    nc.sync.dma_start(out=dst_s2, in_=hsb3[:, :, HS_COLS:wc])

            # ---- phase B: windows ----
            with tc.tile_pool(name="pb_g", bufs=2) as pbg, \
                 tc.tile_pool(name="pb_sm", bufs=2) as pbsm, \
                 tc.tile_pool(name="pb_oh", bufs=2) as pboh, \
                 tc.tile_pool(name="pb_ps", bufs=2, space="PSUM") as pbps, \
                 tc.tile_pool(name="pb_out", bufs=2) as pbout:
                for w in range(nw):
                    cs = slice(w * G, (w + 1) * G)
                    hs_g = pbg.tile([P, G * HS_COLS], f32, tag="hsg")
                    s2_g = pbsm.tile([P, G * HEADS], f32, tag="s2g")
                    for j in range(G):
                        col = w * G + j
                        nc.gpsimd.indirect_dma_start(
                            out=hs_g[:, j * HS_COLS:(j + 1) * HS_COLS],
                            out_offset=None, in_=hs_d[:],
                            in_offset=bass.IndirectOffsetOnAxis(
                                ap=snd_sb[:, col:col + 1], axis=0))
                        nc.gpsimd.indirect_dma_start(
                            out=s2_g[:, j * HEADS:(j + 1) * HEADS],
                            out_offset=None, in_=s2_d[:],
                            in_offset=bass.IndirectOffsetOnAxis(
                                ap=s2i_sb[:, col:col + 1], axis=0))

                    hs_g3 = hs_g[:].rearrange("p (j c) -> p j c", c=HS_COLS)
                    logit = pbsm.tile([P, G * HEADS], f32, tag="logit")
                    lg3 = logit[:].rearrange("p (j h) -> p j h", h=HEADS)
                    nc.vector.tensor_add(
                        out=lg3, in0=hs_g3[:, :, HU:HS_COLS],
                        in1=s2_g[:].rearrange("p (j h) -> p j h", h=HEADS))
                    neg = pbsm.tile([P, G * HEADS], f32, tag="neg")
                    nc.vector.tensor_scalar(
                        out=neg[:], in0=logit[:], scalar1=0.0,
                        scalar2=LEAKY_ALPHA, op0=mybir.AluOpType.min,
                        op1=mybir.AluOpType.mult)
                    lrl = pbsm.tile([P, G * HEADS], f32, tag="lrl")
                    nc.vector.scalar_tensor_tensor(
                        out=lrl[:], in0=logit[:], scalar=0.0, in1=neg[:],
                        op0=mybir.AluOpType.max, op1=mybir.AluOpType.add)
                    expo = pbsm.tile([P, G * HEADS], f32, tag="expo")
                    nc.scalar.activation(
                        out=expo[:], in_=lrl[:],
                        func=mybir.ActivationFunctionType.Exp)

                    rhs = pbg.tile([P, G * HS_COLS], f32, tag="rhs")
                    rhs3 = rhs[:].rearrange("p (j c) -> p j c", c=HS_COLS)
                    ex3 = expo[:].rearrange("p (j h) -> p j h", h=HEADS)
                    nc.vector.tensor_tensor(
                        out=rhs3[:, :, 0:HU].rearrange("p j (h u) -> p j h u",
                                                       u=UNITS),
                        in0=hs_g3[:, :, 0:HU].rearrange("p j (h u) -> p j h u",
                                                        u=UNITS),
                        in1=ex3.broadcast_to([P, G, HEADS, UNITS]),
                        op=mybir.AluOpType.mult)
                    nc.vector.tensor_copy(out=rhs3[:, :, HU:HS_COLS], in_=ex3)

                    onehot = pboh.tile([P, G * P], f32, tag="oh")
                    oh3 = onehot[:].rearrange("p (j c) -> p j c", c=P)
                    nc.vector.tensor_tensor(
                        out=oh3,
                        in0=iota_sb[:].broadcast_to([P, P, G]).rearrange(
                            "p c j -> p j c"),
                        in1=rcvl_sb[:, cs].broadcast_to([P, G, P]),
                        op=mybir.AluOpType.is_equal)

                    ps = pbps.tile([P, HS_COLS], f32, tag="acc")
                    for j in range(G):
                        nc.tensor.matmul(
                            out=ps[:],
                            lhsT=onehot[:, j * P:(j + 1) * P],
                            rhs=rhs[:, j * HS_COLS:(j + 1) * HS_COLS],
                            start=(j == 0), stop=(j == G - 1))

                    if debug and w == 0:
                        nc.sync.dma_start(out=dbg_hsg[:], in_=hs_g[:])
                        nc.sync.dma_start(out=dbg_oh[:], in_=onehot[:])
                        nc.sync.dma_start(out=dbg_rhs[:], in_=rhs[:])
                        ps_copy = pbout.tile([P, HS_COLS], f32, tag="pscopy")
                        nc.vector.tensor_copy(out=ps_copy[:], in_=ps[:])
                        nc.sync.dma_start(out=dbg_ps[:], in_=ps_copy[:])

                    recip = pbout.tile([P, HEADS], f32, tag="recip")
                    nc.vector.reciprocal(out=recip[:], in_=ps[:, HU:HS_COLS])
                    osb = pbout.tile([P, HU], f32, tag="osb")
                    nc.vector.tensor_tensor(
                        out=osb[:].rearrange("p (h u) -> p h u", u=UNITS),
                        in0=ps[:, 0:HU].rearrange("p (h u) -> p h u", u=UNITS),
                        in1=recip[:].broadcast_to([P, HEADS, UNITS]),
                        op=mybir.AluOpType.mult)
                    nc.sync.dma_start(
                        out=out_d[w * P:(w + 1) * P, :], in_=osb[:])

    nc.compile()
    return nc


def _run(nc, per_core, n_cores):
    from concourse import bass_utils

    want_trace = bool(os.environ.get("GAT_TRACE"))
    res = bass_utils.run_bass_kernel_spmd(
        nc, per_core, core_ids=list(range(n_cores)), trace=want_trace)
    return res


def kernel(x, edge_index, W, att_w1, att_w2, n_cores=8, _return_results=False):
    x = np.asarray(x)
    edge_index = np.asarray(edge_index)
    W = np.asarray(W)
    att_w1 = np.asarray(att_w1)
    att_w2 = np.asarray(att_w2)

    n_nodes, in_feat = x.shape
    host, per_core = _build_host_data(x, edge_index, W, att_w1, att_w2, n_cores)
    nc = _build_bass(n_nodes, host["npad"], host["nw"], in_feat)
    res = _run(nc, per_core, n_cores)

    nw = host["nw"]
    out = np.zeros((n_nodes, HU), dtype=np.float32)
    for w, (n0, cnt, e0, e1) in enumerate(host["windows"]):
        c, s = divmod(w, nw)
        staged = res.results[c]["staged"]
        out[n0:n0 + cnt] = staged[s * P:s * P + cnt]
    out[host["deg"] == 0] = 0.0
    if _return_results:
        return out, res
    return out
